# revision 1
# baseline (speedup 1.0000x reference)
"""KronyMLP Trainium2 kernel.

Math (per the reference):
    kr1 = kron(c_fc_1 [1536,32], c_fc_2 [1,12])   -> [1536, 384]
    kr2 = kron(c_proj_1 [32,1536], c_proj_2 [12,1]) -> [384, 1536]
    out = gelu_exact(x @ kr1) @ kr2               x: [16, 4096, 1536] f32

Strategy:
  - Host: materialize kr1/kr2 (tiny), shard x data-parallel over batch across
    8 cores (2 batches = 8192 tokens per core), replicate weights.
  - Device (per core): tile tokens in macro-tiles of 512.
      x natural-layout DMA in -> PE transpose (fp32r, via identity) -> x^T in
      SBUF -> MM1 (lhsT=kr1 chunks, rhs=x^T) accumulating over d-chunks into
      PSUM -> exact-erf Gelu on ScalarE (PSUM->SBUF, h^T layout) ->
      MM2 (lhsT=gelu(h^T) token-columns, rhs=kr2) -> PSUM [tokens, d_out]
      natural layout -> copy -> DMA out.
  - All matmuls run in float32r (full fp32 storage; reduced-precision multiply
    at 1 cycle/row for moving dim >= 256 vs 4 cycles/row for exact fp32).
    Set KRONY_MM_DT=f32 to force exact-fp32 matmuls.
"""

import os
import numpy as np

B, S, D = 16, 4096, 1536
H = 384
N_CORES = 8
T_PER_CORE = (B // N_CORES) * S  # 8192
TN = 512  # tokens per macro tile
P = 128

_BUILT = {}


def _build(T, mm_dt_name):
    import concourse.bacc as bacc
    import concourse.mybir as mybir
    from concourse.bass import ts
    from concourse.tile import TileContext

    f32 = mybir.dt.float32
    mm_dt = {"f32r": mybir.dt.float32r, "f32": mybir.dt.float32}[mm_dt_name]

    DC = D // P        # 12 d-model chunks
    HC = H // P        # 3 hidden chunks
    NO = D // 512      # 3 output column chunks
    n_macro = T // TN
    T4 = TN // P       # 4 token sub-tiles per macro

    nc = bacc.Bacc(None, target_bir_lowering=False, debug=False)
    x_d = nc.declare_dram_parameter("x", [T, D], mm_dt, isOutput=False)
    kr1_d = nc.declare_dram_parameter("kr1", [P, DC, H], mm_dt, isOutput=False)
    kr2_d = nc.declare_dram_parameter("kr2", [P, HC, D], mm_dt, isOutput=False)
    id_d = nc.declare_dram_parameter("ident", [P, P], mm_dt, isOutput=False)
    out_d = nc.declare_dram_parameter("out", [T, D], f32, isOutput=True)

    with TileContext(nc) as tc:
        with (
            tc.tile_pool(name="const", bufs=1) as cpool,
            tc.tile_pool(name="xin", bufs=3) as xpool,
            tc.tile_pool(name="xt", bufs=2) as xtpool,
            tc.tile_pool(name="gh", bufs=2) as ghpool,
            tc.tile_pool(name="outp", bufs=3) as opool,
            tc.tile_pool(name="ps_t", bufs=1, space="PSUM") as pst,
            tc.tile_pool(name="ps_h", bufs=1, space="PSUM") as psh,
            tc.tile_pool(name="ps_o", bufs=2, space="PSUM") as pso,
        ):
            ident = cpool.tile([P, P], mm_dt)
            nc.sync.dma_start(out=ident[:], in_=id_d[:, :])
            kr1_sb = cpool.tile([P, DC, H], mm_dt)
            nc.sync.dma_start(out=kr1_sb[:], in_=kr1_d[:, :, :])
            kr2_sb = cpool.tile([P, HC, D], mm_dt)
            nc.sync.dma_start(out=kr2_sb[:], in_=kr2_d[:, :, :])

            for mi in range(n_macro):
                t0 = mi * TN
                # ---- load + transpose x: build x^T [D-chunks, TN] ----
                # One DVE copy per d-chunk writes the exact region MM1 reads,
                # keeping per-matmul sync-wait counts within HW limits.
                xt = xtpool.tile([P, DC, TN], mm_dt)
                xins = []
                for t4 in range(T4):
                    xin = xpool.tile([P, D], mm_dt, tag=f"xin{t4}")
                    nc.sync.dma_start(
                        out=xin[:], in_=x_d[t0 + t4 * P : t0 + (t4 + 1) * P, :]
                    )
                    xins.append(xin)
                for d in range(DC):
                    ps = pst.tile([P, T4, P], mm_dt)
                    for t4 in range(T4):
                        nc.tensor.transpose(
                            ps[:, t4, :], xins[t4][:, ts(d, P)], ident[:]
                        )
                    nc.vector.tensor_copy(out=xt[:, d, :], in_=ps[:])
                # ---- MM1 + gelu: h^T = gelu(kr1^T-chunks @ x^T) ----
                gh = ghpool.tile([P, HC, TN], mm_dt)
                for m in range(HC):
                    ph = psh.tile([P, TN], f32)
                    for d in range(DC):
                        nc.tensor.matmul(
                            ph[:],
                            lhsT=kr1_sb[:, d, ts(m, P)],
                            rhs=xt[:, d, :],
                            start=(d == 0),
                            stop=(d == DC - 1),
                        )
                    nc.scalar.activation(
                        out=gh[:, m, :],
                        in_=ph[:],
                        func=mybir.ActivationFunctionType.Gelu,
                    )
                # ---- MM2: out[tokens, D] = gelu(h)^T-cols @ kr2 ----
                for t4 in range(T4):
                    po = pso.tile([P, NO, 512], f32)
                    for k in range(HC):
                        for n in range(NO):
                            nc.tensor.matmul(
                                po[:, n, :],
                                lhsT=gh[:, k, ts(t4, P)],
                                rhs=kr2_sb[:, k, ts(n, 512)],
                                start=(k == 0),
                                stop=(k == HC - 1),
                            )
                    orow = opool.tile([P, D], f32)
                    for n in range(NO):
                        nc.vector.tensor_copy(out=orow[:, ts(n, 512)], in_=po[:, n, :])
                    nc.sync.dma_start(
                        out=out_d[t0 + t4 * P : t0 + (t4 + 1) * P, :], in_=orow[:]
                    )
    nc.finalize()
    return nc


def get_nc(T=T_PER_CORE, mm_dt_name=None):
    if mm_dt_name is None:
        mm_dt_name = os.environ.get("KRONY_MM_DT", "f32r")
    key = (T, mm_dt_name)
    if key not in _BUILT:
        _BUILT[key] = _build(T, mm_dt_name)
    return _BUILT[key]


def _host_weights(c_fc_1, c_fc_2, c_proj_1, c_proj_2):
    kr1 = np.kron(np.asarray(c_fc_1, np.float32), np.asarray(c_fc_2, np.float32))
    kr2 = np.kron(np.asarray(c_proj_1, np.float32), np.asarray(c_proj_2, np.float32))
    # device layouts: kr1 [1536,384] -> [128, 12, 384]; kr2 [384,1536] -> [128, 3, 1536]
    kr1_dev = np.ascontiguousarray(
        kr1.reshape(D // P, P, H).transpose(1, 0, 2)
    )
    kr2_dev = np.ascontiguousarray(
        kr2.reshape(H // P, P, D).transpose(1, 0, 2)
    )
    return kr1_dev, kr2_dev


def run_sharded(x, c_fc_1, c_fc_2, c_proj_1, c_proj_2, T=T_PER_CORE, trace=False,
                tmpdir=None):
    from concourse.bass_utils import run_bass_kernel_spmd

    x = np.asarray(x, np.float32)
    n_tok = x.shape[0] * x.shape[1] * 1  # flattened below
    kr1_dev, kr2_dev = _host_weights(c_fc_1, c_fc_2, c_proj_1, c_proj_2)
    ident = np.eye(P, dtype=np.float32)

    xf = x.reshape(-1, D)
    assert xf.shape[0] == N_CORES * T, (xf.shape, T)
    in_maps = [
        {
            "x": np.ascontiguousarray(xf[i * T : (i + 1) * T]),
            "kr1": kr1_dev,
            "kr2": kr2_dev,
            "ident": ident,
        }
        for i in range(N_CORES)
    ]
    nc = get_nc(T)
    res = run_bass_kernel_spmd(
        nc, in_maps, list(range(N_CORES)), trace=trace, tmpdir=tmpdir
    )
    outs = [res.results[i]["out"] for i in range(N_CORES)]
    full = np.concatenate(outs, axis=0).reshape(x.shape)
    return full, res


def kernel(x, c_fc_1, c_fc_2, c_proj_1, c_proj_2):
    out, _ = run_sharded(x, c_fc_1, c_fc_2, c_proj_1, c_proj_2)
    return out.astype(np.float32)



# revision 3
# speedup vs baseline: 1.0044x; 1.0044x over previous
"""KronyMLP Trainium2 kernel (final).

Reference math:
    kr1 = kron(c_fc_1 [1536,32], c_fc_2 [1,12])   -> [1536, 384]
    kr2 = kron(c_proj_1 [32,1536], c_proj_2 [12,1]) -> [384, 1536]
    out = gelu_exact(x @ kr1) @ kr2               x: [16, 4096, 1536] f32

The Kronecker structure is factored instead of materialized:
    h[t, 12i+j] = u[t, i] * B[j]      with u = x @ c_fc_1   (1536 -> 32)
    out[t, :]   = g2[t, :] @ c_proj_1 with g2[t, i] = sum_j gelu(h)[t, 12i+j]*c2[j]
cutting matmul work ~5x vs materializing kron().

Per 512-token macro tile (3-stage software pipeline across macros;
PE stream per step: MM1(s) -> E(s-1) -> MM2(s-2) -> R(s-1) so PE never
waits on the DVE/ACT copies):
    MM1  u^T[128,512] = x-chunks @ c_fc_1-chunks     bf16, 4x col-tiled
    E    h^T[384,512] = E_ext^T @ u^T                f32r (E_ext sums the
         4 col-group partials and expands 32->384; B/c2 must stay f32:
         sum_j B_j c2_j cancels ~6x below its terms' RMS, so bf16 weights
         there shift the output ~3%)
    ACT  gh = gelu(h^T)            exact-erf Gelu, PSUM->SBUF f32r
    R    g2^T[32,512] = sum_m R_m^T @ gh_m           f32r, + bf16 identity
         matmul regroups the 2nd token half to partitions 64-95
         (fp32 matmuls cannot col-tile; bf16 ones can)
    MM2  out[128tok,512] per (half, n)               bf16, 2-way row-tiled;
         PSUM drains alternate DVE/ACT and cast f32->bf16
Sharding: data-parallel over batch (8192 tokens/core, 8 cores), weights
replicated. All I/O in bf16 (host casts + pre-transposes x so no on-device
transpose); 24 MiB in + 24 MiB out per core ~= the DMA floor.
"""

import numpy as np
import ml_dtypes

BF16 = ml_dtypes.bfloat16

B, S, D = 16, 4096, 1536
HP = 32          # factored hidden (columns of c_fc_1)
J = 12           # kron expansion factor (columns of c_fc_2)
H = HP * J       # 384
N_CORES = 8
T_PER_CORE = (B * S) // N_CORES   # 8192
TN = 512         # tokens per macro tile
NM = T_PER_CORE // TN             # 16 macro tiles
P = 128
DC = D // P      # 12 d-model chunks
HC = H // P      # 3 hidden chunks
NO = D // 512    # 3 output column chunks
GQ = 4           # macros per input DMA group

_BUILT = {}


def _build():
    import concourse.bacc as bacc
    import concourse.mybir as mybir
    from concourse.bass import ts
    from concourse.tile import TileContext

    f32 = mybir.dt.float32
    bf = mybir.dt.bfloat16

    nc = bacc.Bacc(None, target_bir_lowering=False, debug=False)
    x_d = nc.declare_dram_parameter("x", [P, NM, DC, TN], bf, isOutput=False)
    w1_d = nc.declare_dram_parameter("w1", [P, DC, HP], bf, isOutput=False)
    f32r = mybir.dt.float32r
    e_d = nc.declare_dram_parameter("eE", [P, H], f32r, isOutput=False)
    r_d = nc.declare_dram_parameter("rR", [P, HC, HP], f32r, isOutput=False)
    cp1_d = nc.declare_dram_parameter("cp1", [P, D], bf, isOutput=False)
    id_d = nc.declare_dram_parameter("id32", [HP, HP], bf, isOutput=False)
    out_d = nc.declare_dram_parameter("out", [NM, P, 4, D], bf, isOutput=True)

    with TileContext(nc) as tc:
        with (
            tc.tile_pool(name="const", bufs=1) as cpool,
            tc.tile_pool(name="xin", bufs=2) as xpool,
            tc.tile_pool(name="u", bufs=2) as upool,
            tc.tile_pool(name="gh", bufs=2) as ghpool,
            tc.tile_pool(name="g2", bufs=2) as g2pool,
            tc.tile_pool(name="outp", bufs=2) as opool,
            tc.tile_pool(name="ps_u", bufs=1, space="PSUM") as psu,
            tc.tile_pool(name="ps_gh", bufs=2, space="PSUM") as psgh,
            tc.tile_pool(name="ps_g2", bufs=1, space="PSUM") as psg2,
            tc.tile_pool(name="ps_o", bufs=2, space="PSUM") as pso,
        ):
            w1_sb = cpool.tile([P, DC, HP], bf)
            nc.sync.dma_start(out=w1_sb[:], in_=w1_d[:, :, :])
            e_sb = cpool.tile([P, H], f32r)
            nc.sync.dma_start(out=e_sb[:], in_=e_d[:, :])
            r_sb = cpool.tile([P, HC, HP], f32r)
            nc.sync.dma_start(out=r_sb[:], in_=r_d[:, :, :])
            cp1_sb = cpool.tile([P, D], bf)
            nc.sync.dma_start(out=cp1_sb[:], in_=cp1_d[:, :])
            id_sb = cpool.tile([HP, HP], bf)
            nc.sync.dma_start(out=id_sb[:], in_=id_d[:, :])

            xts = {}
            us = {}
            NG = NM // GQ

            def issue_group(G):
                # one 1.5 MiB DMA per macro slice: first MM1 of the group can
                # start ~5us after its slice lands instead of waiting 6 MiB.
                # Macro 0's slice is further split by chunk-rounds so round 0
                # can start after ~0.5 MiB.
                xt = xpool.tile([P, GQ, DC, TN], bf, tag="xt")
                for g in range(GQ):
                    if G == 0 and g == 0:
                        for cr in range(3):
                            nc.sync.dma_start(
                                out=xt[:, 0, 4 * cr : 4 * (cr + 1), :],
                                in_=x_d[:, 0, 4 * cr : 4 * (cr + 1), :],
                            )
                    else:
                        nc.sync.dma_start(
                            out=xt[:, g, :, :], in_=x_d[:, G * GQ + g, :, :]
                        )
                xts[G] = xt

            def front(mi):
                # input DMA (prefetched one group ahead) + MM1 + u copy
                g = mi % GQ
                if g == 0:
                    G = mi // GQ
                    if G == 0:
                        issue_group(0)
                    if G + 1 < NG:
                        issue_group(G + 1)
                xt = xts[mi // GQ]
                pu = psu.tile([P, TN], f32, tag="pu")
                for r in range(3):
                    for cg in range(4):
                        c = 4 * r + cg
                        nc.tensor.matmul(
                            pu[32 * cg : 32 * (cg + 1), :],
                            lhsT=w1_sb[:, c, :],
                            rhs=xt[:, g, c, :],
                            start=(r == 0),
                            stop=(r == 2),
                            tile_position=(0, 32 * cg),
                        )
                u_sb = upool.tile([P, TN], f32r, tag="u")
                nc.vector.tensor_copy(out=u_sb[:], in_=pu[:])
                us[mi] = u_sb
                if g == GQ - 1:
                    xts.pop(mi // GQ, None)

            ghs = {}
            g2s = {}
            pgh_live = {}

            def midE(mi):
                # E expand (PE) + gelu (ACT); emitted before backMM2 so the
                # gelus sit ahead of the drain copies in the ACT queue
                u_sb = us.pop(mi)
                gh_sb = ghpool.tile([P, HC, TN], f32r, tag="gh")
                pghs = []
                for m in range(HC):
                    pgh = psgh.tile([P, TN], f32, tag="pgh")
                    nc.tensor.matmul(
                        pgh[:],
                        lhsT=e_sb[:, ts(m, P)],
                        rhs=u_sb[:],
                        start=True,
                        stop=True,
                    )
                    pghs.append(pgh)
                for m in range(HC):
                    nc.scalar.activation(
                        out=gh_sb[:, m, :],
                        in_=pghs[m][:],
                        func=mybir.ActivationFunctionType.Gelu,
                    )
                ghs[mi] = gh_sb

            def midR(mi):
                # R reduce (f32r) + bf16 identity regroup of the second token
                # half to partitions 64-95 (fp32 matmuls cannot col-tile)
                gh_sb = ghs.pop(mi)
                pg2 = psg2.tile([P, TN], f32, tag="pg2")
                for m in range(HC):
                    nc.tensor.matmul(
                        pg2[0:HP, :],
                        lhsT=r_sb[:, m, :],
                        rhs=gh_sb[:, m, :],
                        start=(m == 0),
                        stop=(m == HC - 1),
                    )
                g2_sb = g2pool.tile([P, TN], bf, tag="g2")
                nc.vector.tensor_copy(out=g2_sb[0:HP, :], in_=pg2[0:HP, :])
                nc.tensor.matmul(
                    pg2[64 : 64 + HP, 0 : TN // 2],
                    lhsT=id_sb[:],
                    rhs=g2_sb[0:HP, ts(1, TN // 2)],
                    start=True,
                    stop=True,
                    tile_position=(0, 64),
                )
                nc.vector.tensor_copy(
                    out=g2_sb[64 : 64 + HP, 0 : TN // 2],
                    in_=pg2[64 : 64 + HP, 0 : TN // 2],
                )
                g2s[mi] = g2_sb

            def backMM2(mi):
                # MM2: 2-way row-tiled; drains alternate DVE/ACT
                g2_sb = g2s.pop(mi)
                obuf = opool.tile([P, 4, D], bf, tag="obuf")
                k = 0
                for p2 in range(2):
                    for n in range(NO):
                        po = pso.tile([P, 2, 512], f32, tag="po")
                        for th in range(2):
                            nc.tensor.matmul(
                                po[:, th, :],
                                lhsT=g2_sb[64 * th : 64 * th + HP, ts(p2, P)],
                                rhs=cp1_sb[64 * th : 64 * th + HP, ts(n, 512)],
                                start=True,
                                stop=True,
                                tile_position=(64 * th, 0),
                            )
                        dst = obuf[:, p2 : p2 + 3 : 2, ts(n, 512)]
                        if k % 2 == 0:
                            nc.vector.tensor_copy(out=dst, in_=po[:])
                        else:
                            nc.scalar.activation(
                                out=dst,
                                in_=po[:],
                                func=mybir.ActivationFunctionType.Copy,
                            )
                        k += 1
                nc.sync.dma_start(out=out_d[mi, :, :, :], in_=obuf[:])

            for step in range(NM + 2):
                if step < NM:
                    front(step)
                if 1 <= step <= NM:
                    midE(step - 1)
                if step >= 2:
                    backMM2(step - 2)
                if 1 <= step <= NM:
                    midR(step - 1)

    nc.finalize()
    return nc


def get_nc():
    if "nc" not in _BUILT:
        _BUILT["nc"] = _build()
    return _BUILT["nc"]


def _host_prep_x(x):
    """x [B,S,D] f32 -> per-core [128, NM, DC, TN] bf16, pre-transposed.

    Device moving index k = 128*g + pp maps to token tau = 512*mi + 4*pp + g
    (so the output DMA writes 12 KiB contiguous per partition).
    """
    xf = np.asarray(x, np.float32).reshape(N_CORES, T_PER_CORE, D)
    xb = xf.astype(BF16)
    cores = []
    for i in range(N_CORES):
        xc = xb[i].reshape(NM, P, 4, DC, P)  # [mi, pp, g, c, p]
        xt = np.ascontiguousarray(xc.transpose(4, 0, 3, 2, 1)).reshape(
            P, NM, DC, TN
        )  # [p, mi, c, (g,pp)]
        cores.append(xt)
    return cores


def _host_weights(c_fc_1, c_fc_2, c_proj_1, c_proj_2):
    cfc1 = np.asarray(c_fc_1, np.float32)
    cfc2 = np.asarray(c_fc_2, np.float32).reshape(J)
    cp1 = np.asarray(c_proj_1, np.float32)
    cp2 = np.asarray(c_proj_2, np.float32).reshape(J)

    w1 = np.ascontiguousarray(
        cfc1.reshape(DC, P, HP).transpose(1, 0, 2)
    ).astype(BF16)  # [128, 12, 32]

    eE = np.zeros((HP, H), np.float32)
    for i in range(HP):
        eE[i, i * J : (i + 1) * J] = cfc2
    eE = np.tile(eE, (4, 1))  # [128, 384]: sums MM1's 4 col-group partials
    # eE stays f32 (fed to an f32r dram param) - the j-contraction cancels
    # heavily (sum_j B_j c2_j is ~6x below its terms' RMS), so B/c2 must not
    # be rounded to bf16

    rfull = np.zeros((H, HP), np.float32)
    for i in range(HP):
        rfull[i * J : (i + 1) * J, i] = cp2
    rR = np.ascontiguousarray(
        rfull.reshape(HC, P, HP).transpose(1, 0, 2)
    )  # [128, 3, 32] f32

    cp1b = np.ascontiguousarray(np.tile(cp1, (4, 1))).astype(BF16)  # [128, 1536]
    id32 = np.eye(HP, dtype=np.float32).astype(BF16)
    return w1, eE, rR, cp1b, id32


def run_sharded(x, c_fc_1, c_fc_2, c_proj_1, c_proj_2, trace=False, tmpdir=None):
    from concourse.bass_utils import run_bass_kernel_spmd

    w1, eE, rR, cp1b, id32 = _host_weights(c_fc_1, c_fc_2, c_proj_1, c_proj_2)
    xcores = _host_prep_x(x)
    in_maps = [
        {"x": xcores[i], "w1": w1, "eE": eE, "rR": rR, "cp1": cp1b, "id32": id32}
        for i in range(N_CORES)
    ]
    nc = get_nc()
    res = run_bass_kernel_spmd(
        nc, in_maps, list(range(N_CORES)), trace=trace, tmpdir=tmpdir
    )
    outs = [
        np.asarray(res.results[i]["out"]).reshape(T_PER_CORE, D)
        for i in range(N_CORES)
    ]
    full = np.concatenate(outs, axis=0).astype(np.float32).reshape(np.asarray(x).shape)
    return full, res


def kernel(x, c_fc_1, c_fc_2, c_proj_1, c_proj_2):
    out, _ = run_sharded(x, c_fc_1, c_fc_2, c_proj_1, c_proj_2)
    return out.astype(np.float32)
